# revision 1
# baseline (speedup 1.0000x reference)
"""Trainium2 Bass kernel for nn_LoRALinear (quantized linear + LoRA).

reference:
    w_dq = quant_dequant_int8_per_row(weight)          # [out, in]
    out  = x @ w_dq.T + (alpha/r) * (x @ la) @ lb      # [T, out]

Math identity used here:
    out = x @ (w_dq.T + 2.0 * (la @ lb)) = x @ W_eff

The quant-dequant + LoRA fold is cheap elementwise/skinny-matmul host prep;
the 550-GFLOP dense matmul runs on 8 NeuronCores, data-parallel over tokens.

Device kernel (per core, SPMD identical program):
    xt  [4096, 2048] bf16  - token shard, pre-transposed to [K, M]
    w   [4096, 4096] bf16  - W_eff, replicated
    out [2048, 4096] f32
W_eff's integer-quantized part is bf16-rounded; x is bf16-rounded; matmul
accumulates in fp32 PSUM -> ~2-4e-3 relative error vs the f32 reference.
"""

import numpy as np
import ml_dtypes

TOKENS, IN_F, OUT_F, R = 16384, 4096, 4096, 16
N_CORES = 8
TPC = TOKENS // N_CORES  # tokens per core: 2048
SCALING = 2.0  # alpha / r
P = 128
NS = 512  # out_feature stripe (one PSUM bank of f32)

_NC_CACHE = {}


def _build_nc(tpc=TPC, in_f=IN_F, out_f=OUT_F, ns=NS, repeat=1):
    import concourse.mybir as mybir
    import concourse.tile as tile
    from concourse import bacc

    nc = bacc.Bacc("TRN2", target_bir_lowering=False)

    xt = nc.dram_tensor("xt", [in_f, tpc], mybir.dt.bfloat16, kind="ExternalInput")
    w = nc.dram_tensor("w", [in_f, out_f], mybir.dt.bfloat16, kind="ExternalInput")
    out = nc.dram_tensor("out", [tpc, out_f], mybir.dt.float32, kind="ExternalOutput")

    ko_n = in_f // P   # k-outer tiles (32)
    mt_n = tpc // P    # token tiles (16)
    nt_n = out_f // ns  # out_f stripes (8)
    xc = min(2 * P, tpc)  # x fill chunk: 256 tokens (512B DMA lines)
    x_chunks = tpc // xc

    with tile.TileContext(nc) as tc:
        with (
            tc.tile_pool(name="xpool", bufs=1) as xpool,
            tc.tile_pool(name="wpool", bufs=2) as wpool,
            tc.tile_pool(name="opool", bufs=4) as opool,
            tc.tile_pool(name="pspool", bufs=4, space="PSUM") as pspool,
        ):
            # Whole x shard stays resident in SBUF (bf16: 128 KiB/partition).
            x_sb = xpool.tile([P, ko_n, tpc], mybir.dt.bfloat16)
            xt_r = xt.rearrange("(ko p) m -> p ko m", p=P)
            w_r = w.rearrange("(ko p) n -> p ko n", p=P)

            # Stripe 0 of W and the x chunks are interleaved so the first
            # psum group's matmuls wait only on the first chunks, not the
            # whole 21 MB: PE starts ~10us in, DMA streams under compute.
            kc_n = 4  # stripe-0 ko chunks
            kcs = ko_n // kc_n
            w_sb0 = wpool.tile([P, ko_n, ns], mybir.dt.bfloat16, name="w_sb")
            issue = (
                [("w0", 0), ("x", 0), ("w0", 1), ("x", 1), ("w0", 2), ("w0", 3)]
                + [("x", i) for i in range(2, x_chunks)]
            )
            for kind, i in issue:
                if kind == "w0":
                    nc.sync.dma_start(
                        w_sb0[:, i * kcs : (i + 1) * kcs, :],
                        w_r[:, i * kcs : (i + 1) * kcs, 0:ns],
                    )
                else:
                    nc.sync.dma_start(
                        x_sb[:, :, i * xc : (i + 1) * xc],
                        xt_r[:, :, i * xc : (i + 1) * xc],
                    )

            for _rep in range(repeat):  # repeat>1 only for timing calibration
                for n in range(nt_n):
                    if _rep == 0 and n == 0:
                        w_sb = w_sb0
                    else:
                        w_sb = wpool.tile([P, ko_n, ns], mybir.dt.bfloat16, name="w_sb")
                        nc.sync.dma_start(w_sb[:], w_r[:, :, n * ns : (n + 1) * ns])
                    for m in range(mt_n):
                        ps = pspool.tile([P, ns], mybir.dt.float32)
                        for ko in range(ko_n):
                            nc.tensor.matmul(
                                ps[:],
                                x_sb[:, ko, m * P : (m + 1) * P],
                                w_sb[:, ko, :],
                                start=(ko == 0),
                                stop=(ko == ko_n - 1),
                            )
                        o_sb = opool.tile([P, ns], mybir.dt.float32)
                        nc.vector.tensor_copy(o_sb[:], ps[:])
                        nc.sync.dma_start(
                            out[m * P : (m + 1) * P, n * ns : (n + 1) * ns], o_sb[:]
                        )

    nc.finalize()
    return nc


def _host_prep(x, weight, lora_a, lora_b):
    x = np.asarray(x, dtype=np.float32)
    weight = np.asarray(weight, dtype=np.float32)
    la = np.asarray(lora_a, dtype=np.float32)
    lb = np.asarray(lora_b, dtype=np.float32)

    # Symmetric per-row absmax int8 quant-dequant, matching the reference's
    # fp32 elementwise ops bit-for-bit (max/div/round/clip are exact or
    # correctly rounded in IEEE f32 on any backend).
    abs_max = np.max(np.abs(weight), axis=-1, keepdims=True)
    scale = (abs_max / np.float32(127.0)).astype(np.float32)
    wq = np.clip(
        np.round(weight / (scale + np.float32(1e-8))), -128.0, 127.0
    ).astype(np.float32)
    w_dq = wq * scale

    w_eff = w_dq.T + np.float32(SCALING) * (la @ lb)
    w_bf = w_eff.astype(ml_dtypes.bfloat16)

    x_bf = x.astype(ml_dtypes.bfloat16)
    xt_shards = [
        np.ascontiguousarray(x_bf[c * TPC : (c + 1) * TPC].T) for c in range(N_CORES)
    ]
    return xt_shards, np.ascontiguousarray(w_bf)


def kernel(x, weight, lora_a, lora_b):
    from concourse.bass_utils import run_bass_kernel_spmd

    xt_shards, w_bf = _host_prep(x, weight, lora_a, lora_b)

    if "nc" not in _NC_CACHE:
        _NC_CACHE["nc"] = _build_nc()
    nc = _NC_CACHE["nc"]

    in_maps = [{"xt": xt_shards[c], "w": w_bf} for c in range(N_CORES)]
    res = run_bass_kernel_spmd(nc, in_maps, core_ids=list(range(N_CORES)))
    out = np.concatenate([res.results[c]["out"] for c in range(N_CORES)], axis=0)
    return out



# revision 2
# speedup vs baseline: 1.1407x; 1.1407x over previous
"""Trainium2 Bass kernel for nn_LoRALinear (quantized linear + LoRA).

reference:
    w_dq = quant_dequant_int8_per_row(weight)          # [out, in]
    out  = x @ w_dq.T + (alpha/r) * (x @ la) @ lb      # [T, out]

Math identity used here:
    out = x @ (w_dq.T + 2.0 * (la @ lb)) = x @ W_eff

Host prep folds quant-dequant + LoRA into W_eff, then re-quantizes W_eff to
int8 with per-out-feature symmetric scales. The device dequantizes each
512-wide W stripe to bf16 on the vector engine and runs the 550-GFLOP dense
matmul on 8 NeuronCores, data-parallel over tokens. Outputs are written as
bf16 and upcast to f32 on the host.

Per-core device program (SPMD identical):
    xt  [4, 128, 32, 512] bf16  - token shard in quarters, SBUF layout
    wq  [8, 128, 32, 512] int8  - W_eff stripes, replicated
    ws  [128, 8, 512]     f32   - per-out-feature scales (partition-replicated)
    out [2048, 4096]      bf16
Accumulation is fp32 in PSUM; end-to-end rel err vs the f32 reference is
~8e-3 (int8 requant ~0.9% dominates; bf16 x/out rounding ~0.3%).
"""

import numpy as np
import ml_dtypes

TOKENS, IN_F, OUT_F, R = 16384, 4096, 4096, 16
N_CORES = 8
TPC = TOKENS // N_CORES  # tokens per core: 2048
SCALING = 2.0  # alpha / r
P = 128
NS = 512  # out_feature stripe (one PSUM bank of f32)
QN = 4    # x quarters
QT = TPC // QN  # 512 tokens per quarter

_NC_CACHE = {}


def _build_nc(repeat=1):
    import concourse.mybir as mybir
    import concourse.tile as tile
    from concourse import bacc

    nc = bacc.Bacc("TRN2", target_bir_lowering=False)
    ko_n = IN_F // P    # 32
    ns = NS
    nt_n = OUT_F // ns  # 8
    mo_n = QT // P      # 4

    xt = nc.dram_tensor("xt", [QN, P, ko_n, QT], mybir.dt.bfloat16, kind="ExternalInput")
    wq = nc.dram_tensor("wq", [nt_n, P, ko_n, ns], mybir.dt.int8, kind="ExternalInput")
    ws = nc.dram_tensor("ws", [P, nt_n, ns], mybir.dt.float32, kind="ExternalInput")
    out = nc.dram_tensor("out", [TPC, OUT_F], mybir.dt.bfloat16, kind="ExternalOutput")

    n_steps = QN * nt_n
    total_steps = repeat * n_steps

    with tile.TileContext(nc) as tc:
        with (
            tc.tile_pool(name="xpool", bufs=2) as xpool,
            tc.tile_pool(name="wqpool", bufs=2) as wqpool,
            tc.tile_pool(name="wbpool", bufs=2) as wbpool,
            tc.tile_pool(name="wspool", bufs=1) as wspool,
            tc.tile_pool(name="opool", bufs=4) as opool,
            tc.tile_pool(name="pspool", bufs=4, space="PSUM") as pspool,
        ):
            ws_sb = wspool.tile([P, nt_n, ns], mybir.dt.float32)
            nc.sync.dma_start(ws_sb[:], ws[:])

            def load_dequant(n):
                wq_sb = wqpool.tile([P, ko_n, ns], mybir.dt.int8, name="wq_sb")
                nc.sync.dma_start(wq_sb[:], wq[n])
                wb_sb = wbpool.tile([P, ko_n, ns], mybir.dt.bfloat16, name="wb_sb")
                nc.vector.tensor_tensor(
                    wb_sb[:],
                    wq_sb[:],
                    ws_sb[:, n, :][:, None, :].to_broadcast((P, ko_n, ns)),
                    op=mybir.AluOpType.mult,
                )
                return wb_sb

            def load_x(q):
                x_sb = xpool.tile([P, ko_n, QT], mybir.dt.bfloat16, name="x_sb")
                nc.sync.dma_start(x_sb[:], xt[q])
                return x_sb

            x_cur = load_x(0)
            wb_cur = load_dequant(0)
            for rep in range(repeat):
                for q in range(QN):
                    x_next = None
                    for n in range(nt_n):
                        s = rep * n_steps + q * nt_n + n
                        # prefetch next stripe (and next quarter's x) early so
                        # their DMAs sit ahead of this stripe's out-DMAs in
                        # the sync queue
                        wb_next = (
                            load_dequant((n + 1) % nt_n)
                            if s + 1 < total_steps
                            else None
                        )
                        if n == 0 and s + nt_n < total_steps:
                            x_next = load_x((q + 1) % QN)
                        for mo in range(mo_n):
                            ps = pspool.tile([P, ns], mybir.dt.float32)
                            for ko in range(ko_n):
                                nc.tensor.matmul(
                                    ps[:],
                                    x_cur[:, ko, mo * P : (mo + 1) * P],
                                    wb_cur[:, ko, :],
                                    start=(ko == 0),
                                    stop=(ko == ko_n - 1),
                                )
                            o_sb = opool.tile([P, ns], mybir.dt.bfloat16)
                            nc.scalar.copy(o_sb[:], ps[:])
                            m = q * mo_n + mo
                            nc.sync.dma_start(
                                out[m * P : (m + 1) * P, n * ns : (n + 1) * ns],
                                o_sb[:],
                            )
                        if wb_next is not None:
                            wb_cur = wb_next
                    if x_next is not None:
                        x_cur = x_next
    nc.finalize()
    return nc


def _host_prep(x, weight, lora_a, lora_b):
    x = np.asarray(x, dtype=np.float32)
    weight = np.asarray(weight, dtype=np.float32)
    la = np.asarray(lora_a, dtype=np.float32)
    lb = np.asarray(lora_b, dtype=np.float32)

    # Symmetric per-row absmax int8 quant-dequant, matching the reference's
    # fp32 elementwise ops bit-for-bit.
    abs_max = np.max(np.abs(weight), axis=-1, keepdims=True)
    scale = (abs_max / np.float32(127.0)).astype(np.float32)
    wqr = np.clip(
        np.round(weight / (scale + np.float32(1e-8))), -128.0, 127.0
    ).astype(np.float32)
    w_dq = wqr * scale

    w_eff = w_dq.T + np.float32(SCALING) * (la @ lb)  # [in_f, out_f]

    # Requantize W_eff to int8 with per-out-feature scales; the device
    # dequantizes stripes to bf16. Adds ~0.9% rel err (budget is 2e-2).
    am2 = np.max(np.abs(w_eff), axis=0, keepdims=True)
    sc2 = np.maximum(
        (am2 / np.float32(127.0)).astype(np.float32), np.float32(1e-30)
    )
    wq2 = np.clip(np.round(w_eff / sc2), -128, 127).astype(np.int8)

    wq_dev = np.ascontiguousarray(
        wq2.reshape(IN_F // P, P, OUT_F // NS, NS).transpose(2, 1, 0, 3)
    )  # [8, p, ko, 512]
    ws_dev = np.ascontiguousarray(
        np.broadcast_to(sc2.reshape(OUT_F // NS, NS)[None], (P, OUT_F // NS, NS))
    ).astype(np.float32)  # [p, 8, 512]

    x_bf = x.astype(ml_dtypes.bfloat16)
    xs = []
    for c in range(N_CORES):
        sh = np.ascontiguousarray(x_bf[c * TPC : (c + 1) * TPC].T)  # [in_f, tpc]
        a = sh.reshape(IN_F // P, P, QN, QT).transpose(2, 1, 0, 3)  # [q, p, ko, qt]
        xs.append(np.ascontiguousarray(a))
    return xs, wq_dev, ws_dev


def kernel(x, weight, lora_a, lora_b):
    from concourse.bass_utils import run_bass_kernel_spmd

    xs, wq_dev, ws_dev = _host_prep(x, weight, lora_a, lora_b)

    if "nc" not in _NC_CACHE:
        _NC_CACHE["nc"] = _build_nc()
    nc = _NC_CACHE["nc"]

    in_maps = [{"xt": xs[c], "wq": wq_dev, "ws": ws_dev} for c in range(N_CORES)]
    res = run_bass_kernel_spmd(nc, in_maps, core_ids=list(range(N_CORES)))
    out = np.concatenate(
        [res.results[c]["out"].astype(np.float32) for c in range(N_CORES)], axis=0
    )
    return out
